# revision 6
# baseline (speedup 1.0000x reference)
"""Trainium2 Bass kernel for nn_Attend (l2-distance attention with zero-kv).

Reference computation (per b,h):
    k' = [0; k], v' = [0; v]                       (prepend zero kv)
    scores[i,j] = (2 q_i.k'_j - |q_i|^2 - |k'_j|^2) * (D+2)^-0.5
    causal: j <= i+1 in padded index space
    out = softmax(scores) @ v'

Kernel algebra: softmax is invariant to the per-row constant -scale*|q_i|^2,
so with p~[i,j] = exp(2*scale*q_i.k_j) and ek_j = exp(-scale*|k_j|^2) folded
into the PV stationary operand [V*ek | ek] (zero column contributes exp(0)=1
to the denominator only):
    out_i = (sum_j p~ (v_j ek_j)) / (1 + sum_j p~ ek_j)

Layout: scores are computed TRANSPOSED ([kv, q]) so P^T is directly the
moving operand of the PV matmul.  Heads are processed in PAIRS with K=128
(kT2 stacks both heads' k^T; q^T staged BLOCK-DIAGONALLY) to dodge the
half-rate moving-operand streaming at contraction <= 64.

exp is split across two engines to break the ACT bottleneck:
  - ACT: activation Exp (diagonal blocks + ~half the off-diagonal blocks)
  - DVE: Schraudolph bf16 exp: i16 = trunc(s*C1M + C2P) bit-cast to bf16
    approximates exp(2*scale*s) to ~1.8% rms; one tensor_scalar per block.
Causal masking touches only the 128-col mixed band of each diagonal block
(GPSIMD multiply); QK/exp/PV are column-restricted past the band, with the
diagonal blocks processed in DESCENDING r order so the PV accumulation
start/stop flags stay full-width.

Finalize avoids PE transposes: output stays transposed [d, q] on device
(host un-transposes); denominator+1 is broadcast across partitions by a
K=2 fp32r matmul against [den; ones], then DVE reciprocal + multiply.

Host-side prep (make_in_maps): bf16 cast + transposed/block-diagonal input
layouts + the [V|1] PV operand + mask constants.

Sharding: 32 (b,h) pairs -> 4 heads per core, 8 cores, pure data parallel.
"""

import sys

for _p in ("/opt/trn_rl_repo", "/root/.axon_site"):
    if _p not in sys.path:
        sys.path.insert(0, _p)

import numpy as np

B, H, N, D = 2, 16, 2048, 64
NCORES = 8
HPC = (B * H) // NCORES          # heads per core = 4
NPAIRS = HPC // 2
SCALE = float((D + 2) ** -0.5)   # augmented head dim, matches reference
NB = N // 128                    # kv blocks of 128 = 16
NQT = N // 512                   # q tiles of 512 = 4
LOG2E = 1.4426950408889634
C1M = float(2.0 * SCALE * 128.0 * LOG2E)
CSH = 0.0580                     # schraudolph correction (tuned, floor conv)
C2P = float(16256.0 - 128.0 * CSH + 0.5)  # +0.5: int16 convert truncates

_BUILT = {}


def _build(qk_dt="bfloat16", pv_dt="bfloat16", hpc=HPC, n=N):
    """Build + finalize the SPMD Bass program (one core's view)."""
    NB = n // 128
    NQT = n // 512
    import concourse.mybir as mybir
    import concourse.tile as tile
    from concourse import bacc

    f32 = mybir.dt.float32
    f32r = mybir.dt.float32r
    bf16 = mybir.dt.bfloat16
    i16 = mybir.dt.int16
    Exp = mybir.ActivationFunctionType.Exp
    Ln = mybir.ActivationFunctionType.Ln
    Identity = mybir.ActivationFunctionType.Identity
    add = mybir.AluOpType.add
    mult = mybir.AluOpType.mult

    nc = bacc.Bacc("TRN2", target_bir_lowering=False, debug=False, num_swdge_queues=4)
    qtp_p = nc.declare_dram_parameter("qtp", [NPAIRS, 128, n], bf16, isOutput=False)
    kt2_p = nc.declare_dram_parameter("kt2", [NPAIRS, 128, n], bf16, isOutput=False)
    vo_p = nc.declare_dram_parameter("vo", [hpc, 128, NB, 65], bf16, isOutput=False)
    mg_p = nc.declare_dram_parameter("mg", [128, 2, 128], bf16, isOutput=False)
    oneh_p = nc.declare_dram_parameter("oneh", [NQT, NQT, 64], f32r, isOutput=False)
    o_p = nc.declare_dram_parameter("out", [hpc, 64, n], f32, isOutput=True)

    # off-diagonal exp engine schedule: alternate DVE/ACT (tunable ratio)
    DVE_MOD = 2  # every DVE_MOD-th off-diag block goes to DVE... see below

    with tile.TileContext(nc) as tc:
        with (
            tc.tile_pool(name="const", bufs=1) as constp,
            tc.tile_pool(name="kqt", bufs=2) as kqtp,
            tc.tile_pool(name="vop", bufs=2) as vop,
            tc.tile_pool(name="pt", bufs=6) as ptp,
            tc.tile_pool(name="fin", bufs=5) as finp,
            tc.tile_pool(name="nrmp", bufs=2) as nrmp,
            tc.tile_pool(name="densp", bufs=2) as densp,
            tc.tile_pool(name="ps_s", bufs=3, space="PSUM") as ps_s,
                        tc.tile_pool(name="ps_acc", bufs=1, space="PSUM") as ps_acc,
        ):
            mg = constp.tile([128, 2, 128], bf16, tag="mg")
            nc.sync.dma_start(out=mg[:], in_=mg_p[:])
            oneh = constp.tile([NQT, NQT, 64], f32r, tag="oneh")
            nc.sync.dma_start(out=oneh[:], in_=oneh_p[:])

            # ---- load all pairs (ek pre-folded into vo on host) -----
            qTps, kT2s, vos = [], [], {}
            for pair in range(NPAIRS):
                hA, hB = 2 * pair, 2 * pair + 1
                qTp = kqtp.tile([128, n], bf16, tag="qTp", name=f"qTp_{pair}")
                kT2 = kqtp.tile([128, n], bf16, tag="kT2", name=f"kT2_{pair}")
                if pair == 0:
                    hn = n // 2
                    nc.sync.dma_start(out=qTp[:, 0:hn], in_=qtp_p[pair][:, 0:hn])
                    nc.sync.dma_start(out=qTp[:, hn:n], in_=qtp_p[pair][:, hn:n])
                    nc.scalar.dma_start(out=kT2[:, 0:hn], in_=kt2_p[pair][:, 0:hn])
                    nc.scalar.dma_start(
                        out=kT2[:, hn:n], in_=kt2_p[pair][:, hn:n]
                    )
                else:
                    nc.sync.dma_start(out=qTp[:], in_=qtp_p[pair])
                    nc.scalar.dma_start(out=kT2[:], in_=kt2_p[pair])
                qTps.append(qTp)
                kT2s.append(kT2)
                for h in (hA, hB):
                    vo = vop.tile([128, NB, 65], bf16, tag="vo", name=f"vo_{h}")
                    nc.gpsimd.dma_start(out=vo[:], in_=vo_p[h])
                    vos[h] = vo

            # ---- finalize stage 2 ------------------------------------
            # reciprocal of (1+den) via ACT: 1/(1+x) = exp(-ln(1+x));
            # Ln and Exp share the natural_log_exp_and_others table set,
            # so no PE transposes and no table thrash.
            def stage2_phases(pair, half, densM2, accs2):
                hA, hB = 2 * pair, 2 * pair + 1
                tbase = 2 * half
                tag2 = f"{pair}_{half}"
                st = {}

                def phase_b():
                    lnd = densp.tile([2, 1024], f32, tag="lnd", name=f"ln{tag2}")
                    nc.scalar.activation(lnd[:], densM2[:], Ln, bias=1.0)
                    recs2 = densp.tile([2, 1024], f32r, tag="recs2", name=f"rc{tag2}")
                    nc.scalar.activation(recs2[:], lnd[:], Exp, scale=-1.0)
                    st["recs2"] = recs2

                def phase_c():
                    recs2 = st["recs2"]
                    for tl in range(2):
                        t = tbase + tl
                        db = ps_s.tile(
                            [64, 1024], f32, tag="sp", name=f"db{pair}_{t}"
                        )
                        for hh in range(2):
                            nc.tensor.matmul(
                                db[:, 512 * hh : 512 * (hh + 1)],
                                oneh[0:2, tl, :],
                                recs2[:, 512 * hh : 512 * (hh + 1)],
                                start=True,
                                stop=True,
                            )
                        nrm = nrmp.tile([64, 1024], f32, tag="nrm")
                        nc.vector.tensor_mul(nrm[:], accs2[tl][0:64, :], db[:])
                        nc.sync.dma_start(
                            out=o_p[hA][:, 512 * t : 512 * (t + 1)],
                            in_=nrm[:, 0:512],
                        )
                        nc.sync.dma_start(
                            out=o_p[hB][:, 512 * t : 512 * (t + 1)],
                            in_=nrm[:, 512:1024],
                        )

                return [(tag2, phase_b), (tag2, phase_c)]

            # ---- main flash loop ------------------------------------
            offdiag_ctr = 0
            phase_q = []  # pending stage2 phases, run ≤2 per interleave point
            for pair in range(NPAIRS):
                hA, hB = 2 * pair, 2 * pair + 1
                qTp, kT2 = qTps[pair], kT2s[pair]
                voA, voB = vos[hA], vos[hB]

                densMs = [
                    densp.tile([2, 1024], f32, tag="densM", name=f"dM{pair}_{h2}")
                    for h2 in range(2)
                ]
                accs_t = []
                for t in range(NQT):
                    if t == 2:
                        # this pair's first half (t0,t1 dens staged by now)
                        phase_q.extend(
                            stage2_phases(pair, 0, densMs[0], accs_t[0:2])
                        )
                    if phase_q:
                        gid, fn = phase_q.pop(0)
                        fn()
                        if phase_q and phase_q[0][0] != gid:
                            phase_q.pop(0)[1]()
                    nblk = 4 * (t + 1)
                    acc = ps_acc.tile([65, 1024], f32, tag="acc", name=f"ac{pair}_{t}")
                    # natural j order: start=True is full-width (j=0); the
                    # final stop is partial-width (r=3) which is fine —
                    # has_written state is consistent after j=0's full write.
                    # PV is deferred by 2 blocks so the PE FIFO has lookahead
                    # (QK j+1, j+2 run while exp(j) is in flight).
                    pvq = []

                    def emit_pv(jj):
                        rr = jj - 4 * t
                        cc0 = 128 * rr if rr >= 0 else 0
                        ptj = pvq_pt[jj]
                        nc.tensor.matmul(
                            acc[:, cc0:512],
                            voA[:, jj, :],
                            ptj[:, cc0:512],
                            start=(jj == 0),
                            stop=(jj == nblk - 1),
                        )
                        nc.tensor.matmul(
                            acc[:, 512 + cc0 : 1024],
                            voB[:, jj, :],
                            ptj[:, 512 + cc0 : 1024],
                            start=(jj == 0),
                            stop=(jj == nblk - 1),
                        )

                    pvq_pt = {}
                    for j in range(nblk):
                        r = j - 4 * t
                        diag = r >= 0
                        c0 = 128 * r if diag else 0  # column restriction
                        qsA = qTp[0:64, 512 * t + c0 : 512 * (t + 1)]
                        qsB = qTp[64:128, 512 * t + c0 : 512 * (t + 1)]
                        sp = ps_s.tile([128, 1024], f32, tag="sp")
                        nc.tensor.matmul(
                            sp[:, c0:512],
                            kT2[0:64, 128 * j : 128 * (j + 1)],
                            qsA,
                            start=True,
                            stop=True,
                        )
                        nc.tensor.matmul(
                            sp[:, 512 + c0 : 1024],
                            kT2[64:128, 128 * j : 128 * (j + 1)],
                            qsB,
                            start=True,
                            stop=True,
                        )
                        if len(pvq) >= 2:
                            emit_pv(pvq.pop(0))
                        pt = ptp.tile([128, 1024], bf16, tag="pt")
                        pvq_pt[j] = pt
                        sps = sp[:].rearrange("p (h c) -> p h c", h=2)[:, :, c0:512]
                        pts = pt[:].rearrange("p (h c) -> p h c", h=2)[:, :, c0:512]
                        if diag:
                            use_dve = False
                        else:
                            use_dve = (offdiag_ctr * 13) % 24 < 13
                            offdiag_ctr += 1
                        if use_dve:
                            nc.vector.tensor_scalar(
                                pts.bitcast(i16), sps, C1M, C2P, mult, add
                            )
                        else:
                            nc.scalar.activation(
                                pts, sps, Exp, scale=2.0 * SCALE
                            )
                        if diag:
                            # mask the 128-wide mixed band of both heads
                            band = pt[:].rearrange("p (h c) -> p h c", h=2)[
                                :, :, c0 : c0 + 128
                            ]
                            nc.gpsimd.tensor_tensor(band, band, mg[:], mult)
                        pvq.append(j)
                    for jj in pvq:
                        emit_pv(jj)

                    # ---- stash numerators + den row; free acc fast ------
                    # split halves across ACT+DVE so acc frees in ~0.7us
                    accs = finp.tile([65, 1024], f32, tag="accs", name=f"as{pair}_{t}")
                    nc.scalar.copy(accs[:, 0:512], acc[:, 0:512])
                    nc.vector.tensor_copy(accs[:, 512:1024], acc[:, 512:1024])
                    nc.sync.dma_start(
                        out=densMs[t // 2][t % 2 : t % 2 + 1, :], in_=accs[64:65, :]
                    )
                    accs_t.append(accs)

                phase_q.extend(stage2_phases(pair, 1, densMs[1], accs_t[2:4]))

            while phase_q:
                phase_q.pop(0)[1]()

    nc.finalize()
    return nc


def get_program(qk_dt="bfloat16", pv_dt="bfloat16"):
    key = (qk_dt, pv_dt)
    if key not in _BUILT:
        _BUILT[key] = _build(qk_dt, pv_dt)
    return _BUILT[key]


def make_in_maps(q, k, v, pv_dt="bfloat16"):
    """Host-side input staging: bf16 cast + transposed/blocked layouts."""
    import ml_dtypes

    bf = ml_dtypes.bfloat16
    qf = np.asarray(q, dtype=np.float32).reshape(B * H, N, D)
    kf = np.asarray(k, dtype=np.float32).reshape(B * H, N, D)
    vf = np.asarray(v, dtype=np.float32).reshape(B * H, N, D)

    j = np.arange(128)[:, None]
    cc = np.arange(128)[None, :]
    mg1 = (cc >= j).astype(bf)  # [128, 128]
    mg = np.ascontiguousarray(np.broadcast_to(mg1[:, None, :], (128, 2, 128)))
    oneh = np.ascontiguousarray(
        np.broadcast_to(np.eye(NQT, dtype=np.float32)[:, :, None], (NQT, NQT, 64))
    )

    maps = []
    for c in range(NCORES):
        base = c * HPC
        qtp = np.zeros((NPAIRS, 128, N), dtype=bf)
        kt2 = np.empty((NPAIRS, 128, N), dtype=bf)
        vo = np.empty((HPC, 128, NB, 65), dtype=bf)
        for p in range(NPAIRS):
            hA, hB = base + 2 * p, base + 2 * p + 1
            qtp[p, 0:64, :] = qf[hA].T.astype(bf)
            qtp[p, 64:128, :] = qf[hB].T.astype(bf)
            kt2[p, 0:64, :] = kf[hA].T.astype(bf)
            kt2[p, 64:128, :] = kf[hB].T.astype(bf)
        for hh in range(HPC):
            h = base + hh
            # ek from the bf16-rounded k (matches the on-device numerics
            # the QK path sees), folded into [V*ek | ek] on host.
            kh = kf[h].astype(bf).astype(np.float32)  # [N, 64]
            ek = np.exp(-SCALE * np.sum(kh * kh, axis=-1))  # [N]
            ekb = ek.reshape(NB, 128, 1).transpose(1, 0, 2)  # [128, NB, 1]
            vh = vf[h].reshape(NB, 128, D).transpose(1, 0, 2)
            vo[hh, :, :, 0:64] = (vh * ekb).astype(bf)
            vo[hh, :, :, 64] = ekb[:, :, 0].astype(bf)
        maps.append(
            {
                "qtp": qtp,
                "kt2": np.ascontiguousarray(kt2),
                "vo": vo,
                "mg": mg,
                "oneh": oneh,
            }
        )
    return maps


def kernel(q, k, v):
    from concourse.bass_utils import run_bass_kernel_spmd

    nc = get_program()
    maps = make_in_maps(q, k, v)
    res = run_bass_kernel_spmd(nc, maps, list(range(NCORES)))
    out = np.concatenate(
        [res.results[c]["out"] for c in range(NCORES)], axis=0
    )  # [B*H, 64, N]
    return np.ascontiguousarray(out.transpose(0, 2, 1)).reshape(B, H, N, D)



# revision 7
# speedup vs baseline: 1.8337x; 1.8337x over previous
"""Trainium2 Bass kernel for nn_Attend (l2-distance attention with zero-kv).

Reference computation (per b,h):
    k' = [0; k], v' = [0; v]                       (prepend zero kv)
    scores[i,j] = (2 q_i.k'_j - |q_i|^2 - |k'_j|^2) * (D+2)^-0.5
    causal: j <= i+1 in padded index space
    out = softmax(scores) @ v'

Kernel algebra: softmax is invariant to the per-row constant -scale*|q_i|^2,
so with p~[i,j] = exp(2*scale*q_i.k_j) and ek_j = exp(-scale*|k_j|^2) folded
into the PV stationary operand [V*ek | ek] (zero column contributes exp(0)=1
to the denominator only):
    out_i = (sum_j p~ (v_j ek_j)) / (1 + sum_j p~ ek_j)

Layout: scores are computed TRANSPOSED ([kv, q]); heads are processed in
PAIRS, with the two heads' QK matmuls row-tiled onto PE halves (base
partitions 0/64) so they run CONCURRENTLY.

PV uses P^T 128x128 chunks as the STATIONARY operand and [V*ek | ek]
[128, 65] as the MOVING operand, accumulating out[q, 0:65] per q-chunk in
PSUM across kv blocks.  This puts the softmax denominator in PSUM column
64 PER PARTITION (q), so finalize is a tiny DVE chain (add 1, reciprocal,
broadcast multiply) with no PE transposes, no activation-table switches,
and the output leaves the device in natural [q, d] layout.

exp is split across two engines to break the ACT bottleneck:
  - ACT: activation Exp (diagonal blocks + ~half the off-diagonal blocks)
  - DVE: Schraudolph bf16 exp: i16 = trunc(s*C1M + C2P) bit-cast to bf16
    approximates exp(2*scale*s) to ~1.8% rms; one tensor_scalar per block.
Causal masking touches only the 128-col mixed band of each diagonal block
(GPSIMD multiply); QK/exp/PV are column-restricted past the band.

Host-side prep (make_in_maps): bf16 cast + transposed input layouts + the
[V*ek | ek] PV operand (ek computed on host from bf16-rounded k) + mask
constants.

Sharding: 32 (b,h) pairs -> 4 heads per core, 8 cores, pure data parallel.
"""

import sys

for _p in ("/opt/trn_rl_repo", "/root/.axon_site"):
    if _p not in sys.path:
        sys.path.insert(0, _p)

import numpy as np

B, H, N, D = 2, 16, 2048, 64
NCORES = 8
HPC = (B * H) // NCORES          # heads per core = 4
NPAIRS = HPC // 2
SCALE = float((D + 2) ** -0.5)   # augmented head dim, matches reference
NB = N // 128                    # kv blocks of 128 = 16
NQT = N // 512                   # q tiles of 512 = 4
LOG2E = 1.4426950408889634
C1M = float(2.0 * SCALE * 128.0 * LOG2E)
CSH = 0.0580                     # schraudolph correction (tuned, floor conv)
C2P = float(16256.0 - 128.0 * CSH + 0.5)  # +0.5: int16 convert truncates

_BUILT = {}


def _build(qk_dt="bfloat16", pv_dt="bfloat16", hpc=HPC, n=N):
    """Build + finalize the SPMD Bass program (one core's view)."""
    NB = n // 128
    NQT = n // 512
    import concourse.mybir as mybir
    import concourse.tile as tile
    from concourse import bacc

    f32 = mybir.dt.float32
    bf16 = mybir.dt.bfloat16
    i16 = mybir.dt.int16
    Exp = mybir.ActivationFunctionType.Exp
    add = mybir.AluOpType.add
    mult = mybir.AluOpType.mult

    nc = bacc.Bacc("TRN2", target_bir_lowering=False, debug=False, num_swdge_queues=4)
    qtp_p = nc.declare_dram_parameter("qtp", [NPAIRS, 128, n], bf16, isOutput=False)
    kt2_p = nc.declare_dram_parameter("kt2", [NPAIRS, 128, n], bf16, isOutput=False)
    vo_p = nc.declare_dram_parameter("vo", [hpc, 128, NB, 65], bf16, isOutput=False)
    mg_p = nc.declare_dram_parameter("mg", [128, 2, 128], bf16, isOutput=False)
    o_p = nc.declare_dram_parameter("out", [hpc, n, 64], f32, isOutput=True)

    # off-diagonal exp engine schedule: alternate DVE/ACT (tunable ratio)

    with tile.TileContext(nc) as tc:
        with (
            tc.tile_pool(name="const", bufs=1) as constp,
            tc.tile_pool(name="kqt", bufs=2) as kqtp,
            tc.tile_pool(name="vop", bufs=2) as vop,
            tc.tile_pool(name="pt", bufs=6) as ptp,
            tc.tile_pool(name="fin", bufs=3) as finp,
            tc.tile_pool(name="ps_s", bufs=3, space="PSUM") as ps_s,
            tc.tile_pool(name="ps_acc", bufs=2, space="PSUM") as ps_acc,
        ):
            mg = constp.tile([128, 2, 128], bf16, tag="mg")
            nc.sync.dma_start(out=mg[:], in_=mg_p[:])

            # ---- load all pairs (ek pre-folded into vo on host) -----
            qTps, kT2s, vos = [], [], {}
            for pair in range(NPAIRS):
                hA, hB = 2 * pair, 2 * pair + 1
                qTp = kqtp.tile([128, n], bf16, tag="qTp", name=f"qTp_{pair}")
                kT2 = kqtp.tile([128, n], bf16, tag="kT2", name=f"kT2_{pair}")
                if pair == 0:
                    hn = n // 2
                    nc.sync.dma_start(out=qTp[:, 0:hn], in_=qtp_p[pair][:, 0:hn])
                    nc.sync.dma_start(out=qTp[:, hn:n], in_=qtp_p[pair][:, hn:n])
                    nc.scalar.dma_start(out=kT2[:, 0:hn], in_=kt2_p[pair][:, 0:hn])
                    nc.scalar.dma_start(
                        out=kT2[:, hn:n], in_=kt2_p[pair][:, hn:n]
                    )
                else:
                    nc.sync.dma_start(out=qTp[:], in_=qtp_p[pair])
                    nc.scalar.dma_start(out=kT2[:], in_=kt2_p[pair])
                qTps.append(qTp)
                kT2s.append(kT2)
                for h in (hA, hB):
                    vo = vop.tile([128, NB, 65], bf16, tag="vo", name=f"vo_{h}")
                    nc.gpsimd.dma_start(out=vo[:], in_=vo_p[h])
                    vos[h] = vo

            # ---- main flash loop ------------------------------------
            offdiag_ctr = 0
            for pair in range(NPAIRS):
                hA, hB = 2 * pair, 2 * pair + 1
                qTp, kT2 = qTps[pair], kT2s[pair]
                voA, voB = vos[hA], vos[hB]

                for t in range(NQT):
                    nblk = 4 * (t + 1)
                    # per-head accumulators: [q-chunk part, 4 chunks, V|den]
                    # padded to a full PSUM bank so the single start=True
                    # (whole-bank has_written clear) owns the bank.
                    accT = [
                        ps_acc.tile(
                            [128, 4, 65],
                            f32,
                            tag="acc",
                            name=f"ac{pair}_{t}_{h2}",
                            padded_shape=[128, 4, 128],
                        )
                        for h2 in range(2)
                    ]

                    # PV is deferred by 2 blocks so the PE FIFO has
                    # lookahead (QK j+1, j+2 run while exp(j) is in
                    # flight).  Stationary = P^T chunk, moving = vo.
                    pvq = []
                    pvq_pt = {}

                    def emit_pv(jj):
                        rr = jj - 4 * t
                        ptj = pvq_pt[jj]
                        for h2 in range(2):
                            vo = voA if h2 == 0 else voB
                            for qc in range(max(rr, 0), 4):
                                nc.tensor.matmul(
                                    accT[h2][:, qc, :],
                                    ptj[
                                        :,
                                        512 * h2 + 128 * qc : 512 * h2
                                        + 128 * (qc + 1),
                                    ],
                                    vo[:, jj, :],
                                    start=(jj == 0 and qc == 0),
                                    stop=(jj == 4 * t + qc),
                                )

                    for j in range(nblk):
                        r = j - 4 * t
                        diag = r >= 0
                        c0 = 128 * r if diag else 0  # column restriction
                        qsA = qTp[0:64, 512 * t + c0 : 512 * (t + 1)]
                        qsB = qTp[64:128, 512 * t + c0 : 512 * (t + 1)]
                        sp = ps_s.tile([128, 1024], f32, tag="sp")
                        nc.tensor.matmul(
                            sp[:, c0:512],
                            kT2[0:64, 128 * j : 128 * (j + 1)],
                            qsA,
                            start=True,
                            stop=True,
                        )
                        nc.tensor.matmul(
                            sp[:, 512 + c0 : 1024],
                            kT2[64:128, 128 * j : 128 * (j + 1)],
                            qsB,
                            start=True,
                            stop=True,
                        )
                        if len(pvq) >= 2:
                            emit_pv(pvq.pop(0))
                        pt = ptp.tile([128, 1024], bf16, tag="pt")
                        pvq_pt[j] = pt
                        sps = sp[:].rearrange("p (h c) -> p h c", h=2)[:, :, c0:512]
                        pts = pt[:].rearrange("p (h c) -> p h c", h=2)[:, :, c0:512]
                        if diag:
                            use_dve = False
                        else:
                            use_dve = (offdiag_ctr * 13) % 24 < 13
                            offdiag_ctr += 1
                        if use_dve:
                            nc.vector.tensor_scalar(
                                pts.bitcast(i16), sps, C1M, C2P, mult, add
                            )
                        else:
                            nc.scalar.activation(
                                pts, sps, Exp, scale=2.0 * SCALE
                            )
                        if diag:
                            # mask the 128-wide mixed band of both heads
                            band = pt[:].rearrange("p (h c) -> p h c", h=2)[
                                :, :, c0 : c0 + 128
                            ]
                            nc.gpsimd.tensor_tensor(band, band, mg[:], mult)
                        pvq.append(j)
                    for jj in pvq:
                        emit_pv(jj)

                    # ---- finalize: per-partition den -> tiny DVE chain
                    for h2, h in enumerate((hA, hB)):
                        rec = finp.tile(
                            [128, 4, 1], f32, tag="rec", name=f"rc{pair}_{t}_{h2}"
                        )
                        nc.vector.tensor_scalar_add(
                            rec[:, :, 0], accT[h2][:, :, 64], 1.0
                        )
                        nc.vector.reciprocal(rec[:], rec[:])
                        nrm = finp.tile(
                            [128, 4, 64], f32, tag="nrm", name=f"nr{pair}_{t}_{h2}"
                        )
                        recb = rec[:].broadcast_to([128, 4, 64])
                        nc.vector.scalar_tensor_tensor(
                            nrm[:], accT[h2][:, :, 0:64], 1.0, recb, mult, mult
                        )
                        nc.sync.dma_start(
                            out=o_p[h][512 * t : 512 * (t + 1), :].rearrange(
                                "(c p) d -> p c d", p=128
                            ),
                            in_=nrm[:],
                        )

    nc.finalize()
    return nc


def get_program(qk_dt="bfloat16", pv_dt="bfloat16"):
    key = (qk_dt, pv_dt)
    if key not in _BUILT:
        _BUILT[key] = _build(qk_dt, pv_dt)
    return _BUILT[key]


def make_in_maps(q, k, v, pv_dt="bfloat16"):
    """Host-side input staging: bf16 cast + transposed/blocked layouts."""
    import ml_dtypes

    bf = ml_dtypes.bfloat16
    qf = np.asarray(q, dtype=np.float32).reshape(B * H, N, D)
    kf = np.asarray(k, dtype=np.float32).reshape(B * H, N, D)
    vf = np.asarray(v, dtype=np.float32).reshape(B * H, N, D)

    j = np.arange(128)[:, None]
    cc = np.arange(128)[None, :]
    mg1 = (cc >= j).astype(bf)  # [128, 128]
    mg = np.ascontiguousarray(np.broadcast_to(mg1[:, None, :], (128, 2, 128)))

    maps = []
    for c in range(NCORES):
        base = c * HPC
        qtp = np.zeros((NPAIRS, 128, N), dtype=bf)
        kt2 = np.empty((NPAIRS, 128, N), dtype=bf)
        vo = np.empty((HPC, 128, NB, 65), dtype=bf)
        for p in range(NPAIRS):
            hA, hB = base + 2 * p, base + 2 * p + 1
            qtp[p, 0:64, :] = qf[hA].T.astype(bf)
            qtp[p, 64:128, :] = qf[hB].T.astype(bf)
            kt2[p, 0:64, :] = kf[hA].T.astype(bf)
            kt2[p, 64:128, :] = kf[hB].T.astype(bf)
        for hh in range(HPC):
            h = base + hh
            # ek from the bf16-rounded k (matches the on-device numerics
            # the QK path sees), folded into [V*ek | ek] on host.
            kh = kf[h].astype(bf).astype(np.float32)  # [N, 64]
            ek = np.exp(-SCALE * np.sum(kh * kh, axis=-1))  # [N]
            ekb = ek.reshape(NB, 128, 1).transpose(1, 0, 2)  # [128, NB, 1]
            vh = vf[h].reshape(NB, 128, D).transpose(1, 0, 2)
            vo[hh, :, :, 0:64] = (vh * ekb).astype(bf)
            vo[hh, :, :, 64] = ekb[:, :, 0].astype(bf)
        maps.append(
            {
                "qtp": qtp,
                "kt2": np.ascontiguousarray(kt2),
                "vo": vo,
                "mg": mg,
            }
        )
    return maps


def kernel(q, k, v):
    from concourse.bass_utils import run_bass_kernel_spmd

    nc = get_program()
    maps = make_in_maps(q, k, v)
    res = run_bass_kernel_spmd(nc, maps, list(range(NCORES)))
    out = np.concatenate(
        [res.results[c]["out"] for c in range(NCORES)], axis=0
    )  # [B*H, N, 64]
    return np.ascontiguousarray(out).reshape(B, H, N, D)


# revision 13
# speedup vs baseline: 1.9221x; 1.0482x over previous
"""Trainium2 Bass kernel for nn_Attend (l2-distance attention with zero-kv).

Reference computation (per b,h):
    k' = [0; k], v' = [0; v]                       (prepend zero kv)
    scores[i,j] = (2 q_i.k'_j - |q_i|^2 - |k'_j|^2) * (D+2)^-0.5
    causal: j <= i+1 in padded index space
    out = softmax(scores) @ v'

Kernel algebra: softmax is invariant to the per-row constant -scale*|q_i|^2,
so with p~[i,j] = exp(2*scale*q_i.k_j) and ek_j = exp(-scale*|k_j|^2) folded
into the PV stationary operand [V*ek | ek] (zero column contributes exp(0)=1
to the denominator only):
    out_i = (sum_j p~ (v_j ek_j)) / (1 + sum_j p~ ek_j)

Layout: scores are computed TRANSPOSED ([kv, q]); heads are processed in
PAIRS, with the two heads' QK matmuls row-tiled onto PE halves (base
partitions 0/64) so they run CONCURRENTLY.

PV uses P^T 128x128 chunks as the STATIONARY operand and [V*ek | ek]
[128, 65] as the MOVING operand, accumulating out[q, 0:65] per q-chunk in
PSUM across kv blocks.  This puts the softmax denominator in PSUM column
64 PER PARTITION (q), so finalize is a tiny DVE chain (add 1, reciprocal,
broadcast multiply) with no PE transposes, no activation-table switches,
and the output leaves the device in natural [q, d] layout.

exp is split across two engines to break the ACT bottleneck:
  - ACT: activation Exp (diagonal blocks + ~half the off-diagonal blocks)
  - DVE: Schraudolph bf16 exp: i16 = trunc(s*C1M + C2P) bit-cast to bf16
    approximates exp(2*scale*s) to ~1.8% rms; one tensor_scalar per block.
Causal masking touches only the 128-col mixed band of each diagonal block
(GPSIMD multiply); QK/exp/PV are column-restricted past the band.

Host-side prep (make_in_maps): bf16 cast + transposed input layouts + the
[V*ek | ek] PV operand (ek computed on host from bf16-rounded k) + mask
constants.

Sharding: 32 (b,h) pairs -> 4 heads per core, 8 cores, pure data parallel.
"""

import sys

for _p in ("/opt/trn_rl_repo", "/root/.axon_site"):
    if _p not in sys.path:
        sys.path.insert(0, _p)

import numpy as np

B, H, N, D = 2, 16, 2048, 64
NCORES = 8
HPC = (B * H) // NCORES          # heads per core = 4
NPAIRS = HPC // 2
SCALE = float((D + 2) ** -0.5)   # augmented head dim, matches reference
NB = N // 128                    # kv blocks of 128 = 16
NQT = N // 512                   # q tiles of 512 = 4
LOG2E = 1.4426950408889634
C1M = float(2.0 * SCALE * 128.0 * LOG2E)
CSH = 0.0580                     # schraudolph correction (tuned, floor conv)
C2P = float(16256.0 - 128.0 * CSH + 0.5)  # +0.5: int16 convert truncates

_BUILT = {}


def _build(qk_dt="bfloat16", pv_dt="bfloat16", hpc=HPC, n=N):
    """Build + finalize the SPMD Bass program (one core's view)."""
    NB = n // 128
    NQT = n // 512
    import concourse.mybir as mybir
    import concourse.tile as tile
    from concourse import bacc

    f32 = mybir.dt.float32
    bf16 = mybir.dt.bfloat16
    i16 = mybir.dt.int16
    Exp = mybir.ActivationFunctionType.Exp
    add = mybir.AluOpType.add
    mult = mybir.AluOpType.mult

    nc = bacc.Bacc("TRN2", target_bir_lowering=False, debug=False, num_swdge_queues=4)
    qtp_p = nc.declare_dram_parameter("qtp", [NPAIRS, 128, n], bf16, isOutput=False)
    kt2_p = nc.declare_dram_parameter("kt2", [NPAIRS, 128, n], bf16, isOutput=False)
    vo_p = nc.declare_dram_parameter("vo", [hpc, 128, NB, 65], bf16, isOutput=False)
    mg_p = nc.declare_dram_parameter("mg", [128, 2, 128], bf16, isOutput=False)
    o_p = nc.declare_dram_parameter("out", [hpc, n, 64], f32, isOutput=True)

    # off-diagonal exp engine schedule: alternate DVE/ACT (tunable ratio)

    with tile.TileContext(nc) as tc:
        with (
            tc.tile_pool(name="const", bufs=1) as constp,
            tc.tile_pool(name="kqt", bufs=2) as kqtp,
            tc.tile_pool(name="vop", bufs=2) as vop,
            tc.tile_pool(name="pt", bufs=6) as ptp,
            tc.tile_pool(name="fin", bufs=3) as finp,
            tc.tile_pool(name="ps_s", bufs=3, space="PSUM") as ps_s,
            tc.tile_pool(name="ps_acc", bufs=2, space="PSUM") as ps_acc,
        ):
            mg = constp.tile([128, 2, 128], bf16, tag="mg")
            nc.sync.dma_start(out=mg[:], in_=mg_p[:])

            # ---- load all pairs (ek pre-folded into vo on host) -----
            # pair0's q/k staged in small leading chunks so the first QK
            # can start as soon as ~64KB lands; vo loads alternate across
            # the gpsimd/vector queues so head B's PV isn't serialized
            # behind head A's load.
            qTps, kT2s, vos = [], [], {}
            for pair in range(NPAIRS):
                hA, hB = 2 * pair, 2 * pair + 1
                qTp = kqtp.tile([128, n], bf16, tag="qTp", name=f"qTp_{pair}")
                kT2 = kqtp.tile([128, n], bf16, tag="kT2", name=f"kT2_{pair}")
                if pair == 0:
                    cuts = (0, 256, 1024, n)
                    for a, b in zip(cuts[:-1], cuts[1:]):
                        nc.sync.dma_start(out=qTp[:, a:b], in_=qtp_p[pair][:, a:b])
                        nc.scalar.dma_start(out=kT2[:, a:b], in_=kt2_p[pair][:, a:b])
                else:
                    nc.sync.dma_start(out=qTp[:], in_=qtp_p[pair])
                    nc.scalar.dma_start(out=kT2[:], in_=kt2_p[pair])
                qTps.append(qTp)
                kT2s.append(kT2)
                for h in (hA, hB):
                    vos[h] = vop.tile(
                        [128, NB, 65], bf16, tag="vo", name=f"vo_{h}"
                    )
                if pair == 0:
                    # both heads' first 4 kv blocks land first (t=0 PV)
                    for h in (hA, hB):
                        nc.gpsimd.dma_start(
                            out=vos[h][:, 0:4], in_=vo_p[h][:, 0:4]
                        )
                    for h in (hA, hB):
                        nc.gpsimd.dma_start(
                            out=vos[h][:, 4:NB], in_=vo_p[h][:, 4:NB]
                        )
                else:
                    for h in (hA, hB):
                        nc.gpsimd.dma_start(out=vos[h][:], in_=vo_p[h])

            # ---- main flash loop ------------------------------------
            # greedy ACT/DVE balance for exp (diag blocks eligible for
            # DVE-Schraudolph too); finalize DVE ops are deferred and
            # drip-fed between blocks so they never burst-serialize the
            # DVE queue at a tile boundary.
            act_load = 0.0
            dve_load = 0.0
            fin_q = []
            for pair in range(NPAIRS):
                hA, hB = 2 * pair, 2 * pair + 1
                qTp, kT2 = qTps[pair], kT2s[pair]
                voA, voB = vos[hA], vos[hB]

                for t in range(NQT):
                    nblk = 4 * (t + 1)
                    # per-head accumulators: [q-chunk part, 4 chunks, V|den]
                    # padded to a full PSUM bank so the single start=True
                    # (whole-bank has_written clear) owns the bank.
                    accT = [
                        ps_acc.tile(
                            [128, 4, 65],
                            f32,
                            tag="acc",
                            name=f"ac{pair}_{t}_{h2}",
                            padded_shape=[128, 4, 128],
                        )
                        for h2 in range(2)
                    ]

                    # PV is deferred by 2 blocks so the PE FIFO has
                    # lookahead (QK j+1, j+2 run while exp(j) is in
                    # flight).  Stationary = P^T chunk, moving = vo.
                    pvq = []
                    pvq_pt = {}

                    def emit_pv(jj):
                        rr = jj - 4 * t
                        ptj = pvq_pt[jj]
                        for h2 in range(2):
                            vo = voA if h2 == 0 else voB
                            for qc in range(max(rr, 0), 4):
                                nc.tensor.matmul(
                                    accT[h2][:, qc, :],
                                    ptj[
                                        :,
                                        512 * h2 + 128 * qc : 512 * h2
                                        + 128 * (qc + 1),
                                    ],
                                    vo[:, jj, :],
                                    start=(jj == 0 and qc == 0),
                                    stop=(jj == 4 * t + qc),
                                )

                    for j in range(nblk):
                        r = j - 4 * t
                        diag = r >= 0
                        c0 = 128 * r if diag else 0  # column restriction
                        qsA = qTp[0:64, 512 * t + c0 : 512 * (t + 1)]
                        qsB = qTp[64:128, 512 * t + c0 : 512 * (t + 1)]
                        sp = ps_s.tile([128, 1024], f32, tag="sp")
                        nc.tensor.matmul(
                            sp[:, c0:512],
                            kT2[0:64, 128 * j : 128 * (j + 1)],
                            qsA,
                            start=True,
                            stop=True,
                        )
                        nc.tensor.matmul(
                            sp[:, 512 + c0 : 1024],
                            kT2[64:128, 128 * j : 128 * (j + 1)],
                            qsB,
                            start=True,
                            stop=True,
                        )
                        if len(pvq) >= 3:
                            emit_pv(pvq.pop(0))
                        if fin_q:
                            fin_q.pop(0)()
                            if fin_q:
                                fin_q.pop(0)()
                        pt = ptp.tile([128, 1024], bf16, tag="pt")
                        pvq_pt[j] = pt
                        sps = sp[:].rearrange("p (h c) -> p h c", h=2)[:, :, c0:512]
                        pts = pt[:].rearrange("p (h c) -> p h c", h=2)[:, :, c0:512]
                        w = 2 * (512 - c0)  # free-dim per partition
                        ca = (172.0 + w) / 1.2
                        cd = (120.0 + w) / 0.96
                        use_dve = (dve_load + cd) < (act_load + ca)
                        if use_dve:
                            dve_load += cd
                            nc.vector.tensor_scalar(
                                pts.bitcast(i16), sps, C1M, C2P, mult, add
                            )
                        else:
                            act_load += ca
                            nc.scalar.activation(
                                pts, sps, Exp, scale=2.0 * SCALE
                            )
                        if diag:
                            # mask the 128-wide mixed band of both heads
                            band = pt[:].rearrange("p (h c) -> p h c", h=2)[
                                :, :, c0 : c0 + 128
                            ]
                            nc.gpsimd.tensor_tensor(band, band, mg[:], mult)
                        pvq.append(j)
                    for jj in pvq:
                        emit_pv(jj)

                    # ---- finalize: per-partition den -> tiny DVE chain,
                    # emitted lazily (2 ops per subsequent block)
                    def make_fin(pair, t, h2, h, accTs):
                        def fin_a():
                            rec = finp.tile(
                                [128, 4, 1],
                                f32,
                                tag="rec",
                                name=f"rc{pair}_{t}_{h2}",
                            )
                            nc.vector.tensor_scalar_add(
                                rec[:, :, 0], accTs[:, :, 64], 1.0
                            )
                            nc.vector.reciprocal(rec[:], rec[:])
                            st["rec"] = rec

                        def fin_b():
                            nrm = finp.tile(
                                [128, 4, 64],
                                f32,
                                tag="nrm",
                                name=f"nr{pair}_{t}_{h2}",
                            )
                            recb = st["rec"][:].broadcast_to([128, 4, 64])
                            nc.vector.scalar_tensor_tensor(
                                nrm[:], accTs[:, :, 0:64], 1.0, recb, mult, mult
                            )
                            nc.sync.dma_start(
                                out=o_p[h][512 * t : 512 * (t + 1), :].rearrange(
                                    "(c p) d -> p c d", p=128
                                ),
                                in_=nrm[:],
                            )

                        st = {}
                        return [fin_a, fin_b]

                    for h2, h in enumerate((hA, hB)):
                        fin_q.extend(make_fin(pair, t, h2, h, accT[h2]))
                        dve_load += 1100.0

            while fin_q:
                fin_q.pop(0)()

    nc.finalize()
    return nc


def get_program(qk_dt="bfloat16", pv_dt="bfloat16"):
    key = (qk_dt, pv_dt)
    if key not in _BUILT:
        _BUILT[key] = _build(qk_dt, pv_dt)
    return _BUILT[key]


def make_in_maps(q, k, v, pv_dt="bfloat16"):
    """Host-side input staging: bf16 cast + transposed/blocked layouts."""
    import ml_dtypes

    bf = ml_dtypes.bfloat16
    qf = np.asarray(q, dtype=np.float32).reshape(B * H, N, D)
    kf = np.asarray(k, dtype=np.float32).reshape(B * H, N, D)
    vf = np.asarray(v, dtype=np.float32).reshape(B * H, N, D)

    j = np.arange(128)[:, None]
    cc = np.arange(128)[None, :]
    mg1 = (cc >= j).astype(bf)  # [128, 128]
    mg = np.ascontiguousarray(np.broadcast_to(mg1[:, None, :], (128, 2, 128)))

    maps = []
    for c in range(NCORES):
        base = c * HPC
        qtp = np.zeros((NPAIRS, 128, N), dtype=bf)
        kt2 = np.empty((NPAIRS, 128, N), dtype=bf)
        vo = np.empty((HPC, 128, NB, 65), dtype=bf)
        for p in range(NPAIRS):
            hA, hB = base + 2 * p, base + 2 * p + 1
            qtp[p, 0:64, :] = qf[hA].T.astype(bf)
            qtp[p, 64:128, :] = qf[hB].T.astype(bf)
            kt2[p, 0:64, :] = kf[hA].T.astype(bf)
            kt2[p, 64:128, :] = kf[hB].T.astype(bf)
        for hh in range(HPC):
            h = base + hh
            # ek from the bf16-rounded k (matches the on-device numerics
            # the QK path sees), folded into [V*ek | ek] on host.
            kh = kf[h].astype(bf).astype(np.float32)  # [N, 64]
            ek = np.exp(-SCALE * np.sum(kh * kh, axis=-1))  # [N]
            ekb = ek.reshape(NB, 128, 1).transpose(1, 0, 2)  # [128, NB, 1]
            vh = vf[h].reshape(NB, 128, D).transpose(1, 0, 2)
            vo[hh, :, :, 0:64] = (vh * ekb).astype(bf)
            vo[hh, :, :, 64] = ekb[:, :, 0].astype(bf)
        maps.append(
            {
                "qtp": qtp,
                "kt2": np.ascontiguousarray(kt2),
                "vo": vo,
                "mg": mg,
            }
        )
    return maps


def kernel(q, k, v):
    from concourse.bass_utils import run_bass_kernel_spmd

    nc = get_program()
    maps = make_in_maps(q, k, v)
    res = run_bass_kernel_spmd(nc, maps, list(range(NCORES)))
    out = np.concatenate(
        [res.results[c]["out"] for c in range(NCORES)], axis=0
    )  # [B*H, N, 64]
    return np.ascontiguousarray(out).reshape(B, H, N, D)


# revision 16
# speedup vs baseline: 2.0527x; 1.0680x over previous
"""Trainium2 Bass kernel for nn_Attend (l2-distance attention with zero-kv).

Reference computation (per b,h):
    k' = [0; k], v' = [0; v]                       (prepend zero kv)
    scores[i,j] = (2 q_i.k'_j - |q_i|^2 - |k'_j|^2) * (D+2)^-0.5
    causal: j <= i+1 in padded index space
    out = softmax(scores) @ v'

Kernel algebra: softmax is invariant to the per-row constant -scale*|q_i|^2,
so with p~[i,j] = exp(2*scale*q_i.k_j) and ek_j = exp(-scale*|k_j|^2) folded
into the PV stationary operand [V*ek | ek] (zero column contributes exp(0)=1
to the denominator only):
    out_i = (sum_j p~ (v_j ek_j)) / (1 + sum_j p~ ek_j)

Layout: scores are computed TRANSPOSED ([kv, q]); heads are processed in
PAIRS, with the two heads' QK matmuls row-tiled onto PE halves (base
partitions 0/64) so they run CONCURRENTLY.

PV uses P^T 128x128 chunks as the STATIONARY operand and [V*ek | ek]
[128, 65] as the MOVING operand, accumulating out[q, 0:65] per q-chunk in
PSUM across kv blocks.  This puts the softmax denominator in PSUM column
64 PER PARTITION (q), so finalize is a tiny DVE chain (add 1, reciprocal,
broadcast multiply) with no PE transposes, no activation-table switches,
and the output leaves the device in natural [q, d] layout.

exp is split across two engines to break the ACT bottleneck:
  - ACT: activation Exp (diagonal blocks + ~half the off-diagonal blocks)
  - DVE: Schraudolph bf16 exp: i16 = trunc(s*C1M + C2P) bit-cast to bf16
    approximates exp(2*scale*s) to ~1.8% rms; one tensor_scalar per block.
Causal masking touches only the 128-col mixed band of each diagonal block
(GPSIMD multiply); QK/exp/PV are column-restricted past the band.

Host-side prep (make_in_maps): bf16 cast + transposed input layouts + the
[V*ek | ek] PV operand (ek computed on host from bf16-rounded k) + mask
constants.

Sharding: 32 (b,h) pairs -> 4 heads per core, 8 cores, pure data parallel.
"""

import sys

for _p in ("/opt/trn_rl_repo", "/root/.axon_site"):
    if _p not in sys.path:
        sys.path.insert(0, _p)

import numpy as np

B, H, N, D = 2, 16, 2048, 64
NCORES = 8
HPC = (B * H) // NCORES          # heads per core = 4
NPAIRS = HPC // 2
SCALE = float((D + 2) ** -0.5)   # augmented head dim, matches reference
NB = N // 128                    # kv blocks of 128 = 16
NQT = N // 512                   # q tiles of 512 = 4
LOG2E = 1.4426950408889634
C1M = float(2.0 * SCALE * 128.0 * LOG2E)
CSH = 0.0580                     # schraudolph correction (tuned, floor conv)
C2P = float(16256.0 - 128.0 * CSH + 0.5)  # +0.5: int16 convert truncates

_BUILT = {}


def _build(qk_dt="bfloat16", pv_dt="bfloat16", hpc=HPC, n=N):
    """Build + finalize the SPMD Bass program (one core's view)."""
    NB = n // 128
    NQT = n // 512
    import concourse.mybir as mybir
    import concourse.tile as tile
    from concourse import bacc

    f32 = mybir.dt.float32
    bf16 = mybir.dt.bfloat16
    i16 = mybir.dt.int16
    Exp = mybir.ActivationFunctionType.Exp
    add = mybir.AluOpType.add
    mult = mybir.AluOpType.mult

    nc = bacc.Bacc("TRN2", target_bir_lowering=False, debug=False, num_swdge_queues=4)
    qtp_p = nc.declare_dram_parameter("qtp", [NPAIRS, 128, n], bf16, isOutput=False)
    kt2_p = nc.declare_dram_parameter("kt2", [NPAIRS, 128, n], bf16, isOutput=False)
    vo_p = nc.declare_dram_parameter("vo", [hpc, 128, NB, 65], bf16, isOutput=False)
    mg_p = nc.declare_dram_parameter("mg", [128, 2, 128], bf16, isOutput=False)
    o_p = nc.declare_dram_parameter("out", [hpc, n, 64], f32, isOutput=True)

    # off-diagonal exp engine schedule: alternate DVE/ACT (tunable ratio)

    with tile.TileContext(nc) as tc:
        with (
            tc.tile_pool(name="const", bufs=1) as constp,
            tc.tile_pool(name="kqt", bufs=2) as kqtp,
            tc.tile_pool(name="vop", bufs=2) as vop,
            tc.tile_pool(name="pt", bufs=6) as ptp,
            tc.tile_pool(name="fin", bufs=3) as finp,
            tc.tile_pool(name="ps_s", bufs=3, space="PSUM") as ps_s,
            tc.tile_pool(name="ps_acc", bufs=2, space="PSUM") as ps_acc,
        ):
            mg = constp.tile([128, 2, 128], bf16, tag="mg")

            # ---- load all pairs (ek pre-folded into vo on host) -----
            # pair0's q/k staged in 512-col leading chunks (exactly what
            # QK t=0 consumes) so the first matmul starts ~10us in; mg
            # rides the scalar queue between kt2 chunks (needed by the
            # first diag mask, right after the first exp).
            qTps, kT2s, vos = [], [], {}
            for pair in range(NPAIRS):
                hA, hB = 2 * pair, 2 * pair + 1
                qTp = kqtp.tile([128, n], bf16, tag="qTp", name=f"qTp_{pair}")
                kT2 = kqtp.tile([128, n], bf16, tag="kT2", name=f"kT2_{pair}")
                if pair == 0:
                    cuts = (0, 512, 1024, n)
                    for ci, (a, b) in enumerate(zip(cuts[:-1], cuts[1:])):
                        nc.sync.dma_start(out=qTp[:, a:b], in_=qtp_p[pair][:, a:b])
                        nc.scalar.dma_start(out=kT2[:, a:b], in_=kt2_p[pair][:, a:b])
                        if ci == 0:
                            nc.scalar.dma_start(out=mg[:], in_=mg_p[:])
                else:
                    nc.sync.dma_start(out=qTp[:], in_=qtp_p[pair])
                    nc.scalar.dma_start(out=kT2[:], in_=kt2_p[pair])
                qTps.append(qTp)
                kT2s.append(kT2)
                for h in (hA, hB):
                    vos[h] = vop.tile(
                        [128, NB, 65], bf16, tag="vo", name=f"vo_{h}"
                    )
                if pair == 0:
                    # both heads' first 4 kv blocks land first (t=0 PV)
                    for h in (hA, hB):
                        nc.gpsimd.dma_start(
                            out=vos[h][:, 0:4], in_=vo_p[h][:, 0:4]
                        )
                    for h in (hA, hB):
                        nc.gpsimd.dma_start(
                            out=vos[h][:, 4:NB], in_=vo_p[h][:, 4:NB]
                        )
                else:
                    for h in (hA, hB):
                        nc.gpsimd.dma_start(out=vos[h][:], in_=vo_p[h])

            # ---- main flash loop ------------------------------------
            # greedy ACT/DVE balance for exp (diag blocks eligible for
            # DVE-Schraudolph too); finalize DVE ops are deferred and
            # drip-fed between blocks so they never burst-serialize the
            # DVE queue at a tile boundary.
            act_load = 0.0
            dve_load = 0.0
            fin_q = []
            for pair in range(NPAIRS):
                hA, hB = 2 * pair, 2 * pair + 1
                qTp, kT2 = qTps[pair], kT2s[pair]
                voA, voB = vos[hA], vos[hB]

                # pair1 runs tiles big-first so the pair boundary meets a
                # dense 16-block tile (keeps the PE HAM-warm through it)
                t_order = range(NQT) if pair == 0 else range(NQT - 1, -1, -1)
                for t in t_order:
                    nblk = 4 * (t + 1)
                    # per-head accumulators: [q-chunk part, 4 chunks, V|den]
                    # padded to a full PSUM bank so the single start=True
                    # (whole-bank has_written clear) owns the bank.
                    accT = [
                        ps_acc.tile(
                            [128, 4, 65],
                            f32,
                            tag="acc",
                            name=f"ac{pair}_{t}_{h2}",
                            padded_shape=[128, 4, 128],
                        )
                        for h2 in range(2)
                    ]

                    # PV is deferred by 2 blocks so the PE FIFO has
                    # lookahead (QK j+1, j+2 run while exp(j) is in
                    # flight).  Stationary = P^T chunk, moving = vo.
                    pvq = []
                    pvq_pt = {}

                    def emit_pv(jj):
                        rr = jj - 4 * t
                        ptj = pvq_pt[jj]
                        for h2 in range(2):
                            vo = voA if h2 == 0 else voB
                            for qc in range(max(rr, 0), 4):
                                nc.tensor.matmul(
                                    accT[h2][:, qc, :],
                                    ptj[
                                        :,
                                        512 * h2 + 128 * qc : 512 * h2
                                        + 128 * (qc + 1),
                                    ],
                                    vo[:, jj, :],
                                    start=(jj == 0 and qc == 0),
                                    stop=(jj == 4 * t + qc),
                                )

                    for j in range(nblk):
                        r = j - 4 * t
                        diag = r >= 0
                        c0 = 128 * r if diag else 0  # column restriction
                        qsA = qTp[0:64, 512 * t + c0 : 512 * (t + 1)]
                        qsB = qTp[64:128, 512 * t + c0 : 512 * (t + 1)]
                        sp = ps_s.tile([128, 1024], f32, tag="sp")
                        nc.tensor.matmul(
                            sp[:, c0:512],
                            kT2[0:64, 128 * j : 128 * (j + 1)],
                            qsA,
                            start=True,
                            stop=True,
                        )
                        nc.tensor.matmul(
                            sp[:, 512 + c0 : 1024],
                            kT2[64:128, 128 * j : 128 * (j + 1)],
                            qsB,
                            start=True,
                            stop=True,
                        )
                        if len(pvq) >= 3:
                            emit_pv(pvq.pop(0))
                        if fin_q:
                            fin_q.pop(0)()
                            if fin_q:
                                fin_q.pop(0)()
                        pt = ptp.tile([128, 1024], bf16, tag="pt")
                        pvq_pt[j] = pt
                        sps = sp[:].rearrange("p (h c) -> p h c", h=2)[:, :, c0:512]
                        pts = pt[:].rearrange("p (h c) -> p h c", h=2)[:, :, c0:512]
                        w = 2 * (512 - c0)  # free-dim per partition
                        ca = (172.0 + w) / 1.2
                        cd = (120.0 + w) / 0.96
                        use_dve = (dve_load + cd) < (act_load + ca)
                        if use_dve:
                            dve_load += cd
                            nc.vector.tensor_scalar(
                                pts.bitcast(i16), sps, C1M, C2P, mult, add
                            )
                        else:
                            act_load += ca
                            nc.scalar.activation(
                                pts, sps, Exp, scale=2.0 * SCALE
                            )
                        if diag:
                            # mask the 128-wide mixed band of both heads
                            band = pt[:].rearrange("p (h c) -> p h c", h=2)[
                                :, :, c0 : c0 + 128
                            ]
                            nc.gpsimd.tensor_tensor(band, band, mg[:], mult)
                        pvq.append(j)
                    for jj in pvq:
                        emit_pv(jj)

                    # ---- finalize: per-partition den -> tiny DVE chain,
                    # emitted lazily (2 ops per subsequent block)
                    def make_fin(pair, t, h2, h, accTs):
                        def fin_a():
                            rec = finp.tile(
                                [128, 4, 1],
                                f32,
                                tag="rec",
                                name=f"rc{pair}_{t}_{h2}",
                            )
                            nc.vector.tensor_scalar_add(
                                rec[:, :, 0], accTs[:, :, 64], 1.0
                            )
                            nc.vector.reciprocal(rec[:], rec[:])
                            st["rec"] = rec

                        def fin_b():
                            nrm = finp.tile(
                                [128, 4, 64],
                                f32,
                                tag="nrm",
                                name=f"nr{pair}_{t}_{h2}",
                            )
                            recb = st["rec"][:].broadcast_to([128, 4, 64])
                            nc.vector.scalar_tensor_tensor(
                                nrm[:], accTs[:, :, 0:64], 1.0, recb, mult, mult
                            )
                            nc.sync.dma_start(
                                out=o_p[h][512 * t : 512 * (t + 1), :].rearrange(
                                    "(c p) d -> p c d", p=128
                                ),
                                in_=nrm[:],
                            )

                        st = {}
                        return [fin_a, fin_b]

                    for h2, h in enumerate((hA, hB)):
                        fin_q.extend(make_fin(pair, t, h2, h, accT[h2]))
                        dve_load += 600.0

            while fin_q:
                fin_q.pop(0)()

    nc.finalize()
    return nc


def get_program(qk_dt="bfloat16", pv_dt="bfloat16"):
    key = (qk_dt, pv_dt)
    if key not in _BUILT:
        _BUILT[key] = _build(qk_dt, pv_dt)
    return _BUILT[key]


def make_in_maps(q, k, v, pv_dt="bfloat16"):
    """Host-side input staging: bf16 cast + transposed/blocked layouts."""
    import ml_dtypes

    bf = ml_dtypes.bfloat16
    qf = np.asarray(q, dtype=np.float32).reshape(B * H, N, D)
    kf = np.asarray(k, dtype=np.float32).reshape(B * H, N, D)
    vf = np.asarray(v, dtype=np.float32).reshape(B * H, N, D)

    j = np.arange(128)[:, None]
    cc = np.arange(128)[None, :]
    mg1 = (cc >= j).astype(bf)  # [128, 128]
    mg = np.ascontiguousarray(np.broadcast_to(mg1[:, None, :], (128, 2, 128)))

    maps = []
    for c in range(NCORES):
        base = c * HPC
        qtp = np.zeros((NPAIRS, 128, N), dtype=bf)
        kt2 = np.empty((NPAIRS, 128, N), dtype=bf)
        vo = np.empty((HPC, 128, NB, 65), dtype=bf)
        for p in range(NPAIRS):
            hA, hB = base + 2 * p, base + 2 * p + 1
            qtp[p, 0:64, :] = qf[hA].T.astype(bf)
            qtp[p, 64:128, :] = qf[hB].T.astype(bf)
            kt2[p, 0:64, :] = kf[hA].T.astype(bf)
            kt2[p, 64:128, :] = kf[hB].T.astype(bf)
        for hh in range(HPC):
            h = base + hh
            # ek from the bf16-rounded k (matches the on-device numerics
            # the QK path sees), folded into [V*ek | ek] on host.
            kh = kf[h].astype(bf).astype(np.float32)  # [N, 64]
            ek = np.exp(-SCALE * np.sum(kh * kh, axis=-1))  # [N]
            ekb = ek.reshape(NB, 128, 1).transpose(1, 0, 2)  # [128, NB, 1]
            vh = vf[h].reshape(NB, 128, D).transpose(1, 0, 2)
            vo[hh, :, :, 0:64] = (vh * ekb).astype(bf)
            vo[hh, :, :, 64] = ekb[:, :, 0].astype(bf)
        maps.append(
            {
                "qtp": qtp,
                "kt2": np.ascontiguousarray(kt2),
                "vo": vo,
                "mg": mg,
            }
        )
    return maps


def kernel(q, k, v):
    from concourse.bass_utils import run_bass_kernel_spmd

    nc = get_program()
    maps = make_in_maps(q, k, v)
    res = run_bass_kernel_spmd(nc, maps, list(range(NCORES)))
    out = np.concatenate(
        [res.results[c]["out"] for c in range(NCORES)], axis=0
    )  # [B*H, N, 64]
    return np.ascontiguousarray(out).reshape(B, H, N, D)


# revision 22
# speedup vs baseline: 2.1351x; 1.0401x over previous
"""Trainium2 Bass kernel for nn_Attend (l2-distance attention with zero-kv).

Reference computation (per b,h):
    k' = [0; k], v' = [0; v]                       (prepend zero kv)
    scores[i,j] = (2 q_i.k'_j - |q_i|^2 - |k'_j|^2) * (D+2)^-0.5
    causal: j <= i+1 in padded index space
    out = softmax(scores) @ v'

Kernel algebra: softmax is invariant to the per-row constant -scale*|q_i|^2,
so with p~[i,j] = exp(2*scale*q_i.k_j) and ek_j = exp(-scale*|k_j|^2) folded
into the PV stationary operand [V*ek | ek] (zero column contributes exp(0)=1
to the denominator only):
    out_i = (sum_j p~ (v_j ek_j)) / (1 + sum_j p~ ek_j)

Layout: scores are computed TRANSPOSED ([kv, q]); heads are processed in
PAIRS, with the two heads' QK matmuls row-tiled onto PE halves (base
partitions 0/64) so they run CONCURRENTLY.

PV uses P^T 128x128 chunks as the STATIONARY operand and [V*ek | ek]
[128, 65] as the MOVING operand, accumulating out[q, 0:65] per q-chunk in
PSUM across kv blocks.  This puts the softmax denominator in PSUM column
64 PER PARTITION (q), so finalize is a tiny DVE chain (add 1, reciprocal,
broadcast multiply) with no PE transposes, no activation-table switches,
and the output leaves the device in natural [q, d] layout.

exp is split across two engines to break the ACT bottleneck:
  - ACT: activation Exp (diagonal blocks + ~half the off-diagonal blocks)
  - DVE: Schraudolph bf16 exp: i16 = trunc(s*C1M + C2P) bit-cast to bf16
    approximates exp(2*scale*s) to ~1.8% rms; one tensor_scalar per block.
Causal masking touches only the 128-col mixed band of each diagonal block
(GPSIMD multiply); QK/exp/PV are column-restricted past the band.

Host-side prep (make_in_maps): bf16 cast + transposed input layouts + the
[V*ek | ek] PV operand (ek computed on host from bf16-rounded k) + mask
constants.

Sharding: 32 (b,h) pairs -> 4 heads per core, 8 cores, pure data parallel.
"""

import sys

for _p in ("/opt/trn_rl_repo", "/root/.axon_site"):
    if _p not in sys.path:
        sys.path.insert(0, _p)

import numpy as np

B, H, N, D = 2, 16, 2048, 64
NCORES = 8
HPC = (B * H) // NCORES          # heads per core = 4
NPAIRS = HPC // 2
SCALE = float((D + 2) ** -0.5)   # augmented head dim, matches reference
NB = N // 128                    # kv blocks of 128 = 16
NQT = N // 512                   # q tiles of 512 = 4
LOG2E = 1.4426950408889634
C1M = float(2.0 * SCALE * 128.0 * LOG2E)
CSH = 0.0580                     # schraudolph correction (tuned, floor conv)
C2P = float(16256.0 - 128.0 * CSH + 0.5)  # +0.5: int16 convert truncates

_BUILT = {}


def _build(qk_dt="bfloat16", pv_dt="bfloat16", hpc=HPC, n=N):
    """Build + finalize the SPMD Bass program (one core's view)."""
    NB = n // 128
    NQT = n // 512
    import concourse.mybir as mybir
    import concourse.tile as tile
    from concourse import bacc

    f32 = mybir.dt.float32
    bf16 = mybir.dt.bfloat16
    i16 = mybir.dt.int16
    Exp = mybir.ActivationFunctionType.Exp
    add = mybir.AluOpType.add
    mult = mybir.AluOpType.mult

    nc = bacc.Bacc("TRN2", target_bir_lowering=False, debug=False, num_swdge_queues=4)
    qtp_p = nc.declare_dram_parameter("qtp", [NPAIRS, 128, n], bf16, isOutput=False)
    kt2_p = nc.declare_dram_parameter("kt2", [NPAIRS, 128, n], bf16, isOutput=False)
    vo_p = nc.declare_dram_parameter("vo", [hpc, 128, NB, 65], bf16, isOutput=False)
    mg_p = nc.declare_dram_parameter("mg", [128, 2, 128], bf16, isOutput=False)
    o_p = nc.declare_dram_parameter("out", [hpc, n, 64], f32, isOutput=True)

    # off-diagonal exp engine schedule: alternate DVE/ACT (tunable ratio)

    with tile.TileContext(nc) as tc:
        with (
            tc.tile_pool(name="const", bufs=1) as constp,
            tc.tile_pool(name="kqt", bufs=2) as kqtp,
            tc.tile_pool(name="vop", bufs=4) as vop,
            tc.tile_pool(name="pt", bufs=6) as ptp,
            tc.tile_pool(name="fin", bufs=3) as finp,
            tc.tile_pool(name="ps_s", bufs=3, space="PSUM") as ps_s,
            tc.tile_pool(name="ps_acc", bufs=2, space="PSUM") as ps_acc,
        ):
            mg = constp.tile([128, 2, 128], bf16, tag="mg")

            # ---- load all pairs (ek pre-folded into vo on host) -----
            # pair0's q/k staged in 512-col leading chunks (exactly what
            # QK t=0 consumes) so the first matmul starts ~10us in; mg
            # rides the scalar queue between kt2 chunks (needed by the
            # first diag mask, right after the first exp).
            qTps, kT2s, vos = [], [], {}
            for pair in range(NPAIRS):
                hA, hB = 2 * pair, 2 * pair + 1
                qTp = kqtp.tile([128, n], bf16, tag="qTp", name=f"qTp_{pair}")
                kT2 = kqtp.tile([128, n], bf16, tag="kT2", name=f"kT2_{pair}")
                if pair == 0:
                    cuts = (0, 512, 1024, n)
                    for ci, (a, b) in enumerate(zip(cuts[:-1], cuts[1:])):
                        nc.sync.dma_start(out=qTp[:, a:b], in_=qtp_p[pair][:, a:b])
                        nc.scalar.dma_start(out=kT2[:, a:b], in_=kt2_p[pair][:, a:b])
                        if ci == 0:
                            nc.scalar.dma_start(out=mg[:], in_=mg_p[:])
                else:
                    nc.sync.dma_start(out=qTp[:], in_=qtp_p[pair])
                    nc.scalar.dma_start(out=kT2[:], in_=kt2_p[pair])
                qTps.append(qTp)
                kT2s.append(kT2)
                for h in (hA, hB):
                    vos[h] = vop.tile(
                        [128, NB, 65], bf16, tag="vo", name=f"vo_{h}"
                    )
                if pair == 0:
                    # both heads' first 4 kv blocks land first (t=0 PV)
                    for h in (hA, hB):
                        nc.gpsimd.dma_start(
                            out=vos[h][:, 0:4], in_=vo_p[h][:, 0:4]
                        )
                    for h in (hA, hB):
                        nc.gpsimd.dma_start(
                            out=vos[h][:, 4:NB], in_=vo_p[h][:, 4:NB]
                        )
                else:
                    for h in (hA, hB):
                        nc.gpsimd.dma_start(out=vos[h][:], in_=vo_p[h])

            # ---- main flash loop ------------------------------------
            # greedy ACT/DVE balance for exp (diag blocks eligible for
            # DVE-Schraudolph too); finalize DVE ops are deferred and
            # drip-fed between blocks so they never burst-serialize the
            # DVE queue at a tile boundary.
            act_load = 0.0
            dve_load = 0.0
            fin_q = []
            pvq = []
            for pair in range(NPAIRS):
                hA, hB = 2 * pair, 2 * pair + 1
                qTp, kT2 = qTps[pair], kT2s[pair]
                voA, voB = vos[hA], vos[hB]

                # pair1 runs tiles big-first so the pair boundary meets a
                # dense 16-block tile (keeps the PE HAM-warm through it)
                t_order = range(NQT) if pair == 0 else range(NQT - 1, -1, -1)
                for t in t_order:
                    nblk = 4 * (t + 1)
                    # per-head accumulators: [q-chunk part, 4 chunks, V|den]
                    # padded to a full PSUM bank so the single start=True
                    # (whole-bank has_written clear) owns the bank.
                    accT = [
                        ps_acc.tile(
                            [128, 4, 65],
                            f32,
                            tag="acc",
                            name=f"ac{pair}_{t}_{h2}",
                            padded_shape=[128, 4, 128],
                        )
                        for h2 in range(2)
                    ]

                    # PV (stationary = P^T chunk, moving = vo) is deferred
                    # by 4 blocks ACROSS tile boundaries so the PE FIFO
                    # always has real work during each tile's QK/exp
                    # warmup.  Finalize pops wait until the previous
                    # tile's PVs have fully drained (j >= 4) and run
                    # BEFORE this tile's first PV so the acc-slot
                    # write-after-read order is correct.
                    def make_pv(tt, jj, ptj, accTs, voAB):
                        def emit():
                            rr = jj - 4 * tt
                            for h2 in range(2):
                                for qc in range(max(rr, 0), 4):
                                    nc.tensor.matmul(
                                        accTs[h2][:, qc, :],
                                        ptj[
                                            :,
                                            512 * h2 + 128 * qc : 512 * h2
                                            + 128 * (qc + 1),
                                        ],
                                        voAB[h2][:, jj, :],
                                        start=(jj == 0 and qc == 0),
                                        stop=(jj == 4 * tt + qc),
                                    )

                        return emit

                    for j in range(nblk):
                        r = j - 4 * t
                        diag = r >= 0
                        c0 = 128 * r if diag else 0  # column restriction
                        qsA = qTp[0:64, 512 * t + c0 : 512 * (t + 1)]
                        qsB = qTp[64:128, 512 * t + c0 : 512 * (t + 1)]
                        sp = ps_s.tile([128, 1024], f32, tag="sp")
                        nc.tensor.matmul(
                            sp[:, c0:512],
                            kT2[0:64, 128 * j : 128 * (j + 1)],
                            qsA,
                            start=True,
                            stop=True,
                        )
                        nc.tensor.matmul(
                            sp[:, 512 + c0 : 1024],
                            kT2[64:128, 128 * j : 128 * (j + 1)],
                            qsB,
                            start=True,
                            stop=True,
                        )
                        if len(pvq) >= 4:
                            pvq.pop(0)[1]()
                        # all older-tile PVs drained -> safe to emit the
                        # older tiles' finalize before this tile's first PV
                        if fin_q and (not pvq or pvq[0][0] == (pair, t)):
                            while fin_q:
                                fin_q.pop(0)()
                        pt = ptp.tile([128, 1024], bf16, tag="pt")
                        sps = sp[:].rearrange("p (h c) -> p h c", h=2)[:, :, c0:512]
                        pts = pt[:].rearrange("p (h c) -> p h c", h=2)[:, :, c0:512]
                        w = 2 * (512 - c0)  # free-dim per partition
                        ca = (172.0 + w) / 1.2
                        cd = (120.0 + w) / 0.96
                        use_dve = (dve_load + cd) < (act_load + ca)
                        if use_dve:
                            dve_load += cd
                            nc.vector.tensor_scalar(
                                pts.bitcast(i16), sps, C1M, C2P, mult, add
                            )
                        else:
                            act_load += ca
                            nc.scalar.activation(
                                pts, sps, Exp, scale=2.0 * SCALE
                            )
                        if diag:
                            # mask the 128-wide mixed band of both heads
                            band = pt[:].rearrange("p (h c) -> p h c", h=2)[
                                :, :, c0 : c0 + 128
                            ]
                            nc.gpsimd.tensor_tensor(band, band, mg[:], mult)
                        pvq.append(
                            ((pair, t), make_pv(t, j, pt, accT, (voA, voB)))
                        )

                    # ---- finalize: per-partition den -> tiny DVE chain,
                    # emitted lazily (2 ops per subsequent block)
                    def make_fin(pair, t, h2, h, accTs):
                        def fin_a():
                            rec = finp.tile(
                                [128, 4, 1],
                                f32,
                                tag="rec",
                                name=f"rc{pair}_{t}_{h2}",
                            )
                            nc.vector.tensor_scalar_add(
                                rec[:, :, 0], accTs[:, :, 64], 1.0
                            )
                            nc.vector.reciprocal(rec[:], rec[:])
                            st["rec"] = rec

                        def fin_b():
                            nrm = finp.tile(
                                [128, 4, 64],
                                f32,
                                tag="nrm",
                                name=f"nr{pair}_{t}_{h2}",
                            )
                            recb = st["rec"][:].broadcast_to([128, 4, 64])
                            nc.vector.scalar_tensor_tensor(
                                nrm[:], accTs[:, :, 0:64], 1.0, recb, mult, mult
                            )
                            nc.sync.dma_start(
                                out=o_p[h][512 * t : 512 * (t + 1), :].rearrange(
                                    "(c p) d -> p c d", p=128
                                ),
                                in_=nrm[:],
                            )

                        st = {}
                        return [fin_a, fin_b]

                    for h2, h in enumerate((hA, hB)):
                        fin_q.extend(make_fin(pair, t, h2, h, accT[h2]))
                        dve_load += 600.0

            while pvq:
                pvq.pop(0)[1]()
            while fin_q:
                fin_q.pop(0)()

    nc.finalize()
    return nc


def get_program(qk_dt="bfloat16", pv_dt="bfloat16"):
    key = (qk_dt, pv_dt)
    if key not in _BUILT:
        _BUILT[key] = _build(qk_dt, pv_dt)
    return _BUILT[key]


def make_in_maps(q, k, v, pv_dt="bfloat16"):
    """Host-side input staging: bf16 cast + transposed/blocked layouts."""
    import ml_dtypes

    bf = ml_dtypes.bfloat16
    qf = np.asarray(q, dtype=np.float32).reshape(B * H, N, D)
    kf = np.asarray(k, dtype=np.float32).reshape(B * H, N, D)
    vf = np.asarray(v, dtype=np.float32).reshape(B * H, N, D)

    j = np.arange(128)[:, None]
    cc = np.arange(128)[None, :]
    mg1 = (cc >= j).astype(bf)  # [128, 128]
    mg = np.ascontiguousarray(np.broadcast_to(mg1[:, None, :], (128, 2, 128)))

    maps = []
    for c in range(NCORES):
        base = c * HPC
        qtp = np.zeros((NPAIRS, 128, N), dtype=bf)
        kt2 = np.empty((NPAIRS, 128, N), dtype=bf)
        vo = np.empty((HPC, 128, NB, 65), dtype=bf)
        for p in range(NPAIRS):
            hA, hB = base + 2 * p, base + 2 * p + 1
            qtp[p, 0:64, :] = qf[hA].T.astype(bf)
            qtp[p, 64:128, :] = qf[hB].T.astype(bf)
            kt2[p, 0:64, :] = kf[hA].T.astype(bf)
            kt2[p, 64:128, :] = kf[hB].T.astype(bf)
        for hh in range(HPC):
            h = base + hh
            # ek from the bf16-rounded k (matches the on-device numerics
            # the QK path sees), folded into [V*ek | ek] on host.
            kh = kf[h].astype(bf).astype(np.float32)  # [N, 64]
            ek = np.exp(-SCALE * np.sum(kh * kh, axis=-1))  # [N]
            ekb = ek.reshape(NB, 128, 1).transpose(1, 0, 2)  # [128, NB, 1]
            vh = vf[h].reshape(NB, 128, D).transpose(1, 0, 2)
            vo[hh, :, :, 0:64] = (vh * ekb).astype(bf)
            vo[hh, :, :, 64] = ekb[:, :, 0].astype(bf)
        maps.append(
            {
                "qtp": qtp,
                "kt2": np.ascontiguousarray(kt2),
                "vo": vo,
                "mg": mg,
            }
        )
    return maps


def kernel(q, k, v):
    from concourse.bass_utils import run_bass_kernel_spmd

    nc = get_program()
    maps = make_in_maps(q, k, v)
    res = run_bass_kernel_spmd(nc, maps, list(range(NCORES)))
    out = np.concatenate(
        [res.results[c]["out"] for c in range(NCORES)], axis=0
    )  # [B*H, N, 64]
    return np.ascontiguousarray(out).reshape(B, H, N, D)
